# revision 2
# baseline (speedup 1.0000x reference)
"""ConvHex (hex-grid graph conv) Trainium2 Bass kernel.

out[b,o,h] = (Wc@x[b,:,h] + sum_k Wn[:,:,k]@x[b,:,nb[h,k]]*mask) / (1+#valid) + bias

Strategy (8 NeuronCores, data-parallel over batch B=256 -> 32/core):
- x shipped bf16 in two layouts: xr [Hp, 32*64] (rows = all-batch feature
  columns, for the neighbor gather) and xc [16, 128, Hp] (batch-pair tiles
  for the center term; even batch on partitions 0-63, odd on 64-127).
- Neighbor gather: dma_gather(transpose=True) from HBM with FULL 4KB rows
  (one descriptor = one hex column for all 32 batches), one gather per
  (slot, h-chunk), index counts trimmed to ceil(valid/128)*128. h is
  count-sorted so low-count tails drop both gather traffic and matmul N.
- Matmul: 7 contraction chunks (center + 6 neighbor slots), K=64 each,
  row-tiled pairs: even batch on PE rows 0-63 -> psum_e, odd on rows
  64-127 -> psum_o, accumulated over chunks in PSUM (f32); neighbor-slot
  matmul N trimmed to the slot's valid column count.
- Epilogue: DVE multiply by 1/(1+count) broadcast (precomputed on host)
  into a merged per-pair tile, single store per (pair, chunk) alternating
  between the two HWDGE queues (sync/scalar).
"""
import os
import numpy as np
import ml_dtypes

B, C_IN, C_OUT, H, K = 256, 64, 128, 1039, 6
NCORES = 8
BL = B // NCORES            # 32 batches per core
NPAIR = BL // 2             # 16
Hp = H + 1                  # 1040; column H (=1039) is the zero pad column
HCS = [384, 384, 272]       # h-chunks (matmul N / psum bank sized)
HC_OFF = [0, 384, 768]
NHC = len(HCS)
BF16 = ml_dtypes.bfloat16

TRACE = bool(int(os.environ.get("KERNEL_TRACE", "0")))
LAST_RESULT = None

_CACHE = {}


def _gather_plan(nks):
    """per (k, hci): (valid_n, npad) with npad = ceil(valid/128)*128."""
    plan = []
    for k in range(K):
        row = []
        for hci, hn in enumerate(HCS):
            valid = min(hn, max(0, nks[k] - HC_OFF[hci]))
            npad = -(-valid // 128) * 128
            row.append((valid, npad))
        plan.append(row)
    return plan


def _build_program(nks):
    import concourse.mybir as mybir
    import concourse.tile as tile
    from concourse import bacc

    plan = _gather_plan(nks)
    nc = bacc.Bacc(name="convhex")
    dt = mybir.dt
    xr = nc.dram_tensor("xr", [Hp, BL * C_IN], dt.bfloat16, kind="ExternalInput")
    xc = nc.dram_tensor("xc", [NPAIR, 128, Hp], dt.bfloat16, kind="ExternalInput")
    wt = nc.dram_tensor("wt", [128, 7 * 128], dt.bfloat16, kind="ExternalInput")
    inv = nc.dram_tensor("inv", [128, Hp], dt.float32, kind="ExternalInput")
    idxt = nc.dram_tensor("idxt", [128, K, NHC, 24], dt.int16,
                          kind="ExternalInput")
    y = nc.dram_tensor("y", [NPAIR, 128, 2, Hp], dt.bfloat16,
                       kind="ExternalOutput")

    with tile.TileContext(nc) as tc:
        with tc.tile_pool(name="const", bufs=1) as cpool, \
             tc.tile_pool(name="gat", bufs=10) as gpool, \
             tc.tile_pool(name="osb", bufs=3) as opool, \
             tc.tile_pool(name="ps", bufs=2, space="PSUM") as pspool:
            it = cpool.tile([128, K, NHC, 24], dt.int16)
            nc.sync.dma_start(it[:], idxt[:, :, :, :])
            wtile = cpool.tile([128, 7 * 128], dt.bfloat16)
            nc.sync.dma_start(wtile[:], wt[:, :])
            invt = cpool.tile([128, Hp], dt.float32)
            nc.sync.dma_start(invt[:], inv[:, :])
            xcts = []
            for p in range(NPAIR):
                xct = cpool.tile([128, Hp], dt.bfloat16, tag=f"xc{p}",
                                 name=f"xct_{p}")
                nc.scalar.dma_start(xct[:], xc[p, :, :])
                xcts.append(xct)

            for hci, hn in enumerate(HCS):
                off = HC_OFF[hci]
                # gather neighbor slots for this h-chunk (full 4KB rows)
                gts = []
                ks_act = [k for k in range(K) if plan[k][hci][0] > 0]
                for k in range(K):
                    if k not in ks_act:
                        gts.append(None)
                        continue
                    npad = plan[k][hci][1]
                    gt = gpool.tile([128, NPAIR, npad], dt.bfloat16,
                                    tag="g", name=f"g_{hci}_{k}")
                    nc.gpsimd.dma_gather(
                        gt[:], xr[:, :],
                        it[:, k, hci, 0:npad // 16],
                        num_idxs=npad, num_idxs_reg=npad,
                        elem_size=BL * C_IN, elem_step=BL * C_IN,
                        transpose=True,
                    )
                    gts.append(gt)
                for blk in range(NPAIR // 2):
                    ps = []
                    for j in range(2):
                        pse = pspool.tile([128, 384], dt.float32, tag=f"pe{j}",
                                          name=f"pse_{hci}_{blk}_{j}")
                        pso = pspool.tile([128, 384], dt.float32, tag=f"po{j}",
                                          name=f"pso_{hci}_{blk}_{j}")
                        ps.append((pse, pso))
                    # chunk-outer: center, then neighbor slots; within a
                    # chunk, 4 matmuls (2 pairs x even/odd row-tiles)
                    for j in range(2):
                        p = 2 * blk + j
                        pse, pso = ps[j]
                        cstop = len(ks_act) == 0
                        nc.tensor.matmul(pse[:, 0:hn], wtile[0:64, 0:128],
                                         xcts[p][0:64, off:off + hn],
                                         start=True, stop=cstop)
                        nc.tensor.matmul(pso[:, 0:hn], wtile[64:128, 0:128],
                                         xcts[p][64:128, off:off + hn],
                                         start=True, stop=cstop)
                    for k in ks_act:
                        last = k == ks_act[-1]
                        vn = plan[k][hci][0]
                        wk = wtile[:, (k + 1) * 128:(k + 2) * 128]
                        gk = gts[k]
                        for j in range(2):
                            p = 2 * blk + j
                            pse, pso = ps[j]
                            nc.tensor.matmul(pse[:, 0:vn], wk[0:64, :],
                                             gk[0:64, p, 0:vn],
                                             start=False, stop=last)
                            nc.tensor.matmul(pso[:, 0:vn], wk[64:128, :],
                                             gk[64:128, p, 0:vn],
                                             start=False, stop=last)
                    # epilogue: multiply by inv (broadcast along partitions),
                    # merged (even, odd) store per pair
                    hv = min(hn, H - off)   # valid output columns
                    for j in range(2):
                        p = 2 * blk + j
                        pse, pso = ps[j]
                        ot = opool.tile([128, 2, 384], dt.bfloat16, tag=f"o{j}",
                                        name=f"ot_{hci}_{blk}_{j}")
                        nc.vector.tensor_mul(ot[:, 0, 0:hv], pse[:, 0:hv],
                                             invt[:, off:off + hv])
                        nc.vector.tensor_mul(ot[:, 1, 0:hv], pso[:, 0:hv],
                                             invt[:, off:off + hv])
                        eng = nc.sync if p % 2 == 0 else nc.scalar
                        eng.dma_start(y[p, :, :, off:off + hv], ot[:, :, 0:hv])
    nc.finalize()
    return nc


def _wrap_idx(idx_1d):
    """index list -> [128, n/16] int16 wrapped (pos i at partition i%16, slot i//16)."""
    n = idx_1d.shape[0]
    w = idx_1d.reshape(n // 16, 16).T
    return np.tile(w, (8, 1)).astype(np.int16)


def _host_prep(x, neighbors, weight_center, weight_neighbors, bias):
    x = np.asarray(x, np.float32)
    nb = np.asarray(neighbors)
    wc = np.asarray(weight_center, np.float32)
    wn = np.asarray(weight_neighbors, np.float32)
    bias = np.asarray(bias, np.float32)

    mask = nb >= 0
    counts = mask.sum(1)
    perm = np.argsort(-counts, kind="stable")              # h sorted by count desc
    inv = (1.0 / (1.0 + counts[perm])).astype(np.float32)  # [H] permuted order
    invp = np.concatenate([inv, np.ones(Hp - H, np.float32)])
    inv_bcast = np.broadcast_to(invp, (128, Hp)).copy()

    nks = tuple(int((counts > k).sum()) for k in range(K))
    plan = _gather_plan(nks)
    safe = np.where(mask, nb, H).astype(np.int16)[perm]    # [H, K] rows permuted
    colp = np.concatenate([safe, np.full((Hp - H, K), H, np.int16)], axis=0)
    idxt = np.zeros((128, K, NHC, 24), np.int16)
    for k in range(K):
        for hci in range(NHC):
            valid, npad = plan[k][hci]
            if npad == 0:
                continue
            lst = np.full(npad, H, np.int16)
            lst[:valid] = colp[HC_OFF[hci]:HC_OFF[hci] + valid, k]
            idxt[:, k, hci, 0:npad // 16] = _wrap_idx(lst)

    # weights: lhsT [128, 7*128] bf16, chunk c: rows 0-63 = W.T, 64-127 = W.T
    wt = np.zeros((128, 7 * 128), np.float32)
    wt[0:64, 0:128] = wc.T
    wt[64:128, 0:128] = wc.T
    for k in range(K):
        wt[0:64, (k + 1) * 128:(k + 2) * 128] = wn[:, :, k].T
        wt[64:128, (k + 1) * 128:(k + 2) * 128] = wn[:, :, k].T
    wt = wt.astype(BF16)

    xb = x.astype(BF16)                                    # [B, 64, H]
    in_maps = []
    for c in range(NCORES):
        xs = xb[c * BL:(c + 1) * BL]                       # [32, 64, H]
        xrc = np.zeros((Hp, BL, C_IN), BF16)
        xrc[:H] = xs.transpose(2, 0, 1)
        xcc = np.zeros((NPAIR, 128, Hp), BF16)
        xcc[:, 0:64, :H] = xs[0::2][:, :, perm]
        xcc[:, 64:128, :H] = xs[1::2][:, :, perm]
        in_maps.append({
            "xr": xrc.reshape(Hp, BL * C_IN),
            "xc": xcc,
            "wt": wt,
            "inv": inv_bcast,
            "idxt": idxt,
        })
    return in_maps, nks, perm


def kernel(x, neighbors, weight_center, weight_neighbors, bias):
    global LAST_RESULT
    from concourse.bass_utils import run_bass_kernel_spmd

    in_maps, nks, perm = _host_prep(x, neighbors, weight_center,
                                    weight_neighbors, bias)
    if _CACHE.get("key") != nks:
        _CACHE["nc"] = _build_program(nks)
        _CACHE["key"] = nks
    nc = _CACHE["nc"]
    res = run_bass_kernel_spmd(nc, in_maps, core_ids=list(range(NCORES)),
                               trace=TRACE)
    LAST_RESULT = res
    inv_perm = np.empty_like(perm)
    inv_perm[perm] = np.arange(perm.shape[0])
    out = np.empty((B, C_OUT, H), np.float32)
    for c, r in enumerate(res.results):
        yc = np.asarray(r["y"])[:, :, :, :H].astype(np.float32)  # [16,128,2,H]
        out[c * BL:(c + 1) * BL] = (
            yc.transpose(0, 2, 1, 3).reshape(BL, C_OUT, H)[:, :, inv_perm]
        )
    b = np.asarray(bias, np.float32)
    if np.any(b != 0.0):
        # reference adds bias after the divide; device epilogue skips it
        out = out + b[None, :, None]
    return np.ascontiguousarray(out)


# revision 5
# speedup vs baseline: 1.0072x; 1.0072x over previous
"""ConvHex (hex-grid graph conv) Trainium2 Bass kernel.

out[b,o,h] = (Wc@x[b,:,h] + sum_k Wn[:,:,k]@x[b,:,nb[h,k]]*mask) / (1+#valid) + bias

Strategy (8 NeuronCores, data-parallel over batch B=256 -> 32/core):
- x shipped bf16 in two layouts: xr [Hp, 32*64] (rows = all-batch feature
  columns, for the neighbor gather) and xc [16, 128, Hp] (batch-pair tiles
  for the center term; even batch on partitions 0-63, odd on 64-127).
- Neighbor gather: dma_gather(transpose=True) from HBM with FULL 4KB rows
  (one descriptor = one hex column for all 32 batches), one gather per
  (slot, h-chunk), index counts trimmed to ceil(valid/128)*128. h is
  count-sorted so low-count tails drop both gather traffic and matmul N.
- Matmul: 7 contraction chunks (center + 6 neighbor slots), K=64 each,
  row-tiled pairs: even batch on PE rows 0-63 -> psum_e, odd on rows
  64-127 -> psum_o, accumulated over chunks in PSUM (f32); neighbor-slot
  matmul N trimmed to the slot's valid column count.
- Epilogue: DVE multiply by 1/(1+count) broadcast (precomputed on host)
  into a merged per-pair tile, single store per (pair, chunk) alternating
  between the two HWDGE queues (sync/scalar).
"""
import os
import numpy as np
import ml_dtypes

B, C_IN, C_OUT, H, K = 256, 64, 128, 1039, 6
NCORES = 8
BL = B // NCORES            # 32 batches per core
NPAIR = BL // 2             # 16
Hp = H + 1                  # 1040; column H (=1039) is the zero pad column
HCS = [384, 384, 272]       # h-chunks (matmul N / psum bank sized)
HC_OFF = [0, 384, 768]
NHC = len(HCS)
BF16 = ml_dtypes.bfloat16

TRACE = bool(int(os.environ.get("KERNEL_TRACE", "0")))
LAST_RESULT = None

_CACHE = {}


def _gather_plan(nks):
    """per (k, hci): (valid_n, npad) with npad = ceil(valid/128)*128."""
    plan = []
    for k in range(K):
        row = []
        for hci, hn in enumerate(HCS):
            valid = min(hn, max(0, nks[k] - HC_OFF[hci]))
            npad = -(-valid // 128) * 128
            row.append((valid, npad))
        plan.append(row)
    return plan


def _build_program(nks):
    import concourse.mybir as mybir
    import concourse.tile as tile
    from concourse import bacc

    plan = _gather_plan(nks)
    nc = bacc.Bacc(name="convhex", num_swdge_queues=2)
    dt = mybir.dt
    xr = nc.dram_tensor("xr", [Hp, BL * C_IN], dt.bfloat16, kind="ExternalInput")
    xc = nc.dram_tensor("xc", [NPAIR, 128, Hp], dt.bfloat16, kind="ExternalInput")
    wt = nc.dram_tensor("wt", [128, 7 * 128], dt.bfloat16, kind="ExternalInput")
    inv = nc.dram_tensor("inv", [128, Hp], dt.float32, kind="ExternalInput")
    idxt = nc.dram_tensor("idxt", [128, K, NHC, 24], dt.int16,
                          kind="ExternalInput")
    y = nc.dram_tensor("y", [NPAIR, 128, 2, Hp], dt.bfloat16,
                       kind="ExternalOutput")

    with tile.TileContext(nc) as tc:
        with tc.tile_pool(name="const", bufs=1) as cpool, \
             tc.tile_pool(name="gat", bufs=10) as gpool, \
             tc.tile_pool(name="osb", bufs=3) as opool, \
             tc.tile_pool(name="ps", bufs=2, space="PSUM") as pspool:
            it = cpool.tile([128, K, NHC, 24], dt.int16)
            nc.sync.dma_start(it[:], idxt[:, :, :, :])
            wtile = cpool.tile([128, 7 * 128], dt.bfloat16)
            nc.sync.dma_start(wtile[:], wt[:, :])
            invt = cpool.tile([128, Hp], dt.float32)
            nc.sync.dma_start(invt[:], inv[:, :])
            xcts = [None] * NPAIR

            def load_xc(p):
                xct = cpool.tile([128, Hp], dt.bfloat16, tag=f"xc{p}",
                                 name=f"xct_{p}")
                nc.scalar.dma_start(xct[:], xc[p, :, :])
                xcts[p] = xct

            gq = 0
            for hci, hn in enumerate(HCS):
                off = HC_OFF[hci]
                # gather neighbor slots for this h-chunk (full 4KB rows)
                gts = []
                ks_act = [k for k in range(K) if plan[k][hci][0] > 0]
                for k in range(K):
                    if k not in ks_act:
                        gts.append(None)
                        continue
                    npad = plan[k][hci][1]
                    gt = gpool.tile([128, NPAIR, npad], dt.bfloat16,
                                    tag="g", name=f"g_{hci}_{k}")
                    nc.gpsimd.dma_gather(
                        gt[:], xr[:, :],
                        it[:, k, hci, 0:npad // 16],
                        num_idxs=npad, num_idxs_reg=npad,
                        elem_size=BL * C_IN, elem_step=BL * C_IN,
                        transpose=True, queue_num=gq,
                    )
                    gq = 1 - gq
                    gts.append(gt)
                if hci == 0:
                    # deferred past the chunk-0 gathers so their issue
                    # doesn't delay the first dma_gather dispatch
                    for p in range(NPAIR):
                        load_xc(p)
                for blk in range(NPAIR // 2):
                    ps = []
                    for j in range(2):
                        pse = pspool.tile([128, 384], dt.float32, tag=f"pe{j}",
                                          name=f"pse_{hci}_{blk}_{j}")
                        pso = pspool.tile([128, 384], dt.float32, tag=f"po{j}",
                                          name=f"pso_{hci}_{blk}_{j}")
                        ps.append((pse, pso))
                    # chunk-outer: center, then neighbor slots; within a
                    # chunk, 4 matmuls (2 pairs x even/odd row-tiles)
                    for j in range(2):
                        p = 2 * blk + j
                        pse, pso = ps[j]
                        cstop = len(ks_act) == 0
                        nc.tensor.matmul(pse[:, 0:hn], wtile[0:64, 0:128],
                                         xcts[p][0:64, off:off + hn],
                                         start=True, stop=cstop)
                        nc.tensor.matmul(pso[:, 0:hn], wtile[64:128, 0:128],
                                         xcts[p][64:128, off:off + hn],
                                         start=True, stop=cstop)
                    for k in ks_act:
                        last = k == ks_act[-1]
                        vn = plan[k][hci][0]
                        wk = wtile[:, (k + 1) * 128:(k + 2) * 128]
                        gk = gts[k]
                        for j in range(2):
                            p = 2 * blk + j
                            pse, pso = ps[j]
                            nc.tensor.matmul(pse[:, 0:vn], wk[0:64, :],
                                             gk[0:64, p, 0:vn],
                                             start=False, stop=last)
                            nc.tensor.matmul(pso[:, 0:vn], wk[64:128, :],
                                             gk[64:128, p, 0:vn],
                                             start=False, stop=last)
                    # epilogue: multiply by inv (broadcast along partitions),
                    # merged (even, odd) store per pair
                    hv = min(hn, H - off)   # valid output columns
                    for j in range(2):
                        p = 2 * blk + j
                        pse, pso = ps[j]
                        ot = opool.tile([128, 2, 384], dt.bfloat16, tag=f"o{j}",
                                        name=f"ot_{hci}_{blk}_{j}")
                        nc.vector.tensor_mul(ot[:, 0, 0:hv], pse[:, 0:hv],
                                             invt[:, off:off + hv])
                        nc.vector.tensor_mul(ot[:, 1, 0:hv], pso[:, 0:hv],
                                             invt[:, off:off + hv])
                        eng = nc.sync if p % 2 == 0 else nc.scalar
                        eng.dma_start(y[p, :, :, off:off + hv], ot[:, :, 0:hv])
    nc.finalize()
    return nc


def _wrap_idx(idx_1d):
    """index list -> [128, n/16] int16 wrapped (pos i at partition i%16, slot i//16)."""
    n = idx_1d.shape[0]
    w = idx_1d.reshape(n // 16, 16).T
    return np.tile(w, (8, 1)).astype(np.int16)


def _host_prep(x, neighbors, weight_center, weight_neighbors, bias):
    x = np.asarray(x, np.float32)
    nb = np.asarray(neighbors)
    wc = np.asarray(weight_center, np.float32)
    wn = np.asarray(weight_neighbors, np.float32)
    bias = np.asarray(bias, np.float32)

    mask = nb >= 0
    counts = mask.sum(1)
    perm = np.argsort(-counts, kind="stable")              # h sorted by count desc
    inv = (1.0 / (1.0 + counts[perm])).astype(np.float32)  # [H] permuted order
    invp = np.concatenate([inv, np.ones(Hp - H, np.float32)])
    inv_bcast = np.broadcast_to(invp, (128, Hp)).copy()

    nks = tuple(int((counts > k).sum()) for k in range(K))
    plan = _gather_plan(nks)
    safe = np.where(mask, nb, H).astype(np.int16)[perm]    # [H, K] rows permuted
    colp = np.concatenate([safe, np.full((Hp - H, K), H, np.int16)], axis=0)
    idxt = np.zeros((128, K, NHC, 24), np.int16)
    for k in range(K):
        for hci in range(NHC):
            valid, npad = plan[k][hci]
            if npad == 0:
                continue
            lst = np.full(npad, H, np.int16)
            lst[:valid] = colp[HC_OFF[hci]:HC_OFF[hci] + valid, k]
            idxt[:, k, hci, 0:npad // 16] = _wrap_idx(lst)

    # weights: lhsT [128, 7*128] bf16, chunk c: rows 0-63 = W.T, 64-127 = W.T
    wt = np.zeros((128, 7 * 128), np.float32)
    wt[0:64, 0:128] = wc.T
    wt[64:128, 0:128] = wc.T
    for k in range(K):
        wt[0:64, (k + 1) * 128:(k + 2) * 128] = wn[:, :, k].T
        wt[64:128, (k + 1) * 128:(k + 2) * 128] = wn[:, :, k].T
    wt = wt.astype(BF16)

    xb = x.astype(BF16)                                    # [B, 64, H]
    in_maps = []
    for c in range(NCORES):
        xs = xb[c * BL:(c + 1) * BL]                       # [32, 64, H]
        xrc = np.zeros((Hp, BL, C_IN), BF16)
        xrc[:H] = xs.transpose(2, 0, 1)
        xcc = np.zeros((NPAIR, 128, Hp), BF16)
        xcc[:, 0:64, :H] = xs[0::2][:, :, perm]
        xcc[:, 64:128, :H] = xs[1::2][:, :, perm]
        in_maps.append({
            "xr": xrc.reshape(Hp, BL * C_IN),
            "xc": xcc,
            "wt": wt,
            "inv": inv_bcast,
            "idxt": idxt,
        })
    return in_maps, nks, perm


def kernel(x, neighbors, weight_center, weight_neighbors, bias):
    global LAST_RESULT
    from concourse.bass_utils import run_bass_kernel_spmd

    in_maps, nks, perm = _host_prep(x, neighbors, weight_center,
                                    weight_neighbors, bias)
    if _CACHE.get("key") != nks:
        _CACHE["nc"] = _build_program(nks)
        _CACHE["key"] = nks
    nc = _CACHE["nc"]
    res = run_bass_kernel_spmd(nc, in_maps, core_ids=list(range(NCORES)),
                               trace=TRACE)
    LAST_RESULT = res
    inv_perm = np.empty_like(perm)
    inv_perm[perm] = np.arange(perm.shape[0])
    out = np.empty((B, C_OUT, H), np.float32)
    for c, r in enumerate(res.results):
        yc = np.asarray(r["y"])[:, :, :, :H].astype(np.float32)  # [16,128,2,H]
        out[c * BL:(c + 1) * BL] = (
            yc.transpose(0, 2, 1, 3).reshape(BL, C_OUT, H)[:, :, inv_perm]
        )
    b = np.asarray(bias, np.float32)
    if np.any(b != 0.0):
        # reference adds bias after the divide; device epilogue skips it
        out = out + b[None, :, None]
    return np.ascontiguousarray(out)
